# revision 1
# baseline (speedup 1.0000x reference)
"""Per-sample 21x21 blur (grouped conv, reflect pad) on trn2, 8 NeuronCores.

Problem: input [16, 3, 768, 768] f32, kernel [16, 21, 21] f32 (one blur
kernel per sample, shared across channels), reflect-pad 10, output
[16, 3, 768, 768] f32.

Strategy (data-parallel over batch, 2 samples/core, 6 images/core):
  The conv becomes TensorE matmuls via a Toeplitz factorization over image
  rows: for an output row-block of M rows, the M+20 input rows covering it
  are contracted against a banded [M+20, M] matrix T_dx holding kernel
  column dx on its diagonals; the 21 dx terms accumulate in one PSUM tile
  with the moving operand shifted along the free (column) axis by dx:

    out[y0+m, x0+n] = sum_dx  T_dx[r, m] * pad[y0+r, x0+dx+n]

  PE cost is purely streamed moving columns (1 bf16 col/cycle), i.e.
  21 * 768 columns per row-block set, so the row-block count is what
  matters.  M=108 (K=128, the partition limit) gives 7 full blocks per
  768-row image; the six 12-row remainder strips are packed into 2
  extra block-diagonal sets (4 images + 2 images stacked on partitions),
  for 44 sets/core instead of 48 with uniform M=96.

  Inputs and Toeplitz weights are pre-cast to bf16 on the host (PSUM
  accumulation stays fp32), which keeps the PE on its fast streaming path.
"""
import sys

sys.path.insert(0, "/opt/trn_rl_repo")

import numpy as np
import ml_dtypes

N_CORES = 8
B, C, H, W = 16, 3, 768, 768
KS = 21          # kernel size
PAD = 10         # reflect pad
HP = H + 2 * PAD  # 788
WP = W + 2 * PAD  # 788
MBLK = 108       # output rows per main matmul block
KBLK = 128       # input rows per main block (= partition limit)
YBLKS = H // MBLK  # 7 full blocks per image
MREM = H - YBLKS * MBLK  # 12 remainder rows per image
KREM = MREM + KS - 1     # 32 input rows per remainder strip
NBLK = 384       # legacy constant (timing probes); chunking below uses CHUNKS
CHUNKS = ((0, 512), (512, 256))  # (x0, width) pairs covering 768 cols
SPC = B // N_CORES  # samples per core = 2
IMGS = SPC * C      # images per core = 6
REM_GROUPS = ((0, 1, 2, 3), (4, 5))  # images packed per remainder set

_prog_cache = {}


def build_program(reps=1, loop_reps=1):
    """loop_reps>1 wraps the whole conv in a hardware For_i loop repeating it
    loop_reps times -- used only for timing (constant instruction count)."""
    import contextlib

    import concourse.bacc as bacc
    import concourse.mybir as mybir
    from concourse.tile import TileContext

    nc = bacc.Bacc(None, target_bir_lowering=False)
    x = nc.declare_dram_parameter("x", [IMGS, HP, WP], mybir.dt.bfloat16,
                                  isOutput=False)
    w = nc.declare_dram_parameter("w", [KBLK, SPC * KS, MBLK], mybir.dt.bfloat16,
                                  isOutput=False)
    wr = [
        nc.declare_dram_parameter(
            f"wr{gi}", [len(g) * KREM, KS, len(g) * MREM], mybir.dt.bfloat16,
            isOutput=False,
        )
        for gi, g in enumerate(REM_GROUPS)
    ]
    y = nc.declare_dram_parameter("y", [IMGS, H, W], mybir.dt.float32,
                                  isOutput=True)

    with TileContext(nc) as tc:
        with (
            tc.tile_pool(name="wpool", bufs=1) as wpool,
            tc.tile_pool(name="xpool", bufs=4) as xpool,
            tc.tile_pool(name="opool", bufs=3) as opool,
            tc.tile_pool(name="psum", bufs=8, space="PSUM") as psum_pool,
        ):
            w_sb = wpool.tile([KBLK, SPC * KS, MBLK], mybir.dt.bfloat16)
            nc.sync.dma_start(out=w_sb[:, :, :], in_=w[:, :, :])
            wr_sb = []
            for gi, g in enumerate(REM_GROUPS):
                t = wpool.tile([len(g) * KREM, KS, len(g) * MREM],
                               mybir.dt.bfloat16, tag=f"wr{gi}")
                nc.sync.dma_start(out=t[:, :, :], in_=wr[gi][:, :, :])
                wr_sb.append(t)

            loop_cm = (
                tc.For_i(0, loop_reps, 1) if loop_reps > 1
                else contextlib.nullcontext()
            )
            with loop_cm:
                for _ in range(reps):
                    # main blocks: M=108, K=128
                    for img in range(IMGS):
                        s = img // C
                        for yb in range(YBLKS):
                            x_sb = xpool.tile([KBLK, WP], mybir.dt.bfloat16)
                            nc.sync.dma_start(
                                out=x_sb[:, :],
                                in_=x[img, yb * MBLK : yb * MBLK + KBLK, :],
                            )
                            out_sb = opool.tile([MBLK, W], mybir.dt.float32)
                            for x0, wdt in CHUNKS:
                                ps = psum_pool.tile([MBLK, 512],
                                                    mybir.dt.float32)
                                for dx in range(KS):
                                    nc.tensor.matmul(
                                        ps[:, :wdt],
                                        w_sb[:, s * KS + dx, :],
                                        x_sb[:, x0 + dx : x0 + dx + wdt],
                                        start=(dx == 0),
                                        stop=(dx == KS - 1),
                                    )
                                nc.vector.tensor_copy(
                                    out=out_sb[:, x0 : x0 + wdt], in_=ps[:, :wdt]
                                )
                            nc.sync.dma_start(
                                out=y[img, yb * MBLK : (yb + 1) * MBLK, :],
                                in_=out_sb[:, :],
                            )
                    # remainder strips: images packed on partitions
                    for gi, g in enumerate(REM_GROUPS):
                        ng = len(g)
                        xr_sb = xpool.tile([ng * KREM, WP], mybir.dt.bfloat16,
                                           tag=f"xr{gi}")
                        for i, img in enumerate(g):
                            nc.sync.dma_start(
                                out=xr_sb[i * KREM : (i + 1) * KREM, :],
                                in_=x[img, YBLKS * MBLK :, :],
                            )
                        outr_sb = opool.tile([ng * MREM, W], mybir.dt.float32,
                                             tag=f"or{gi}")
                        for x0, wdt in CHUNKS:
                            ps = psum_pool.tile([ng * MREM, 512],
                                                mybir.dt.float32, tag="ps")
                            for dx in range(KS):
                                nc.tensor.matmul(
                                    ps[:, :wdt],
                                    wr_sb[gi][:, dx, :],
                                    xr_sb[:, x0 + dx : x0 + dx + wdt],
                                    start=(dx == 0),
                                    stop=(dx == KS - 1),
                                )
                            nc.vector.tensor_copy(
                                out=outr_sb[:, x0 : x0 + wdt], in_=ps[:, :wdt]
                            )
                        for i, img in enumerate(g):
                            nc.sync.dma_start(
                                out=y[img, YBLKS * MBLK :, :],
                                in_=outr_sb[i * MREM : (i + 1) * MREM, :],
                            )
    nc.compile()
    return nc


def _band(kern_col, K, M):
    """[K, M] banded Toeplitz: T[m+j, m] = kern_col[j], j in [0,21)."""
    t = np.zeros((K, M), np.float32)
    for m in range(M):
        t[m : m + KS, m] = kern_col
    return t


def _weights(kern_pair):
    """kern_pair [SPC, 21, 21] -> (w_main, [wr per group]) in bf16."""
    wt = np.zeros((KBLK, SPC * KS, MBLK), np.float32)
    for s in range(SPC):
        for dx in range(KS):
            wt[:, s * KS + dx, :] = _band(kern_pair[s, :, dx], KBLK, MBLK)
    wrs = []
    for g in REM_GROUPS:
        ng = len(g)
        wr = np.zeros((ng * KREM, KS, ng * MREM), np.float32)
        for i, img in enumerate(g):
            s = img // C
            for dx in range(KS):
                wr[i * KREM : (i + 1) * KREM, dx,
                   i * MREM : (i + 1) * MREM] = _band(
                    kern_pair[s, :, dx], KREM, MREM)
        wrs.append(wr.astype(ml_dtypes.bfloat16))
    return wt.astype(ml_dtypes.bfloat16), wrs


def make_in_maps(inp, kern):
    pad = np.pad(inp, ((0, 0), (0, 0), (PAD, PAD), (PAD, PAD)), mode="reflect")
    pad_bf = pad.astype(ml_dtypes.bfloat16)
    in_maps = []
    for c in range(N_CORES):
        s0 = c * SPC
        x_core = pad_bf[s0 : s0 + SPC].reshape(IMGS, HP, WP)
        w_core, wr_core = _weights(kern[s0 : s0 + SPC])
        m = {"x": np.ascontiguousarray(x_core), "w": w_core}
        for gi, wr in enumerate(wr_core):
            m[f"wr{gi}"] = wr
        in_maps.append(m)
    return in_maps


def kernel(input, kernel):
    from concourse.bass_utils import run_bass_kernel_spmd

    inp = np.asarray(input, dtype=np.float32)
    kern = np.asarray(kernel, dtype=np.float32)
    in_maps = make_in_maps(inp, kern)

    if "nc" not in _prog_cache:
        _prog_cache["nc"] = build_program()
    nc = _prog_cache["nc"]

    res = run_bass_kernel_spmd(nc, in_maps, list(range(N_CORES)))
    out = np.empty((B, C, H, W), np.float32)
    for c in range(N_CORES):
        out[c * SPC : (c + 1) * SPC] = res.results[c]["y"].reshape(SPC, C, H, W)
    return out



# revision 2
# speedup vs baseline: 2.0789x; 2.0789x over previous
"""Per-sample 21x21 blur (grouped conv, reflect pad) on trn2, 8 NeuronCores.

Problem: input [16, 3, 768, 768] f32, kernel [16, 21, 21] f32 (one blur
kernel per sample, shared across channels), reflect-pad 10, output
[16, 3, 768, 768] f32.

Strategy (data-parallel over batch, 2 samples/core, 6 images/core):

  Space-to-depth matmul formulation.  The padded image (788x788, zero-
  extended to 792x790) is laid out host-side as 12x10 pixel blocks with
  the 120 in-block pixels on SBUF partitions:

      P2D[p=(a',b'), (py, px)] = pad[12*py + a', 10*px + b']

  An output block (ty, tx) needs pad rows [12ty, 12ty+32) and cols
  [10tx, 10tx+30), i.e. pad-blocks (ty+al, tx+be) for al, be in 0..2.
  The conv is then NINE accumulating matmuls with dense-ish 120x120
  stationaries, the moving operand being the natural P2D tensor at the
  nine block offsets -- no im2col, the layout swizzle is free (host):

      out[(a,b), (ty,tx)] = sum_{al,be} W_ab[(a',b'), (a,b)]
                                        * P2D[(a',b'), (ty+al, tx+be)]
      W_ab[(a',b'), (a,b)] = k[12*al + a' - a, 10*be + b' - b]

  PE cost: 9 streamed columns per 120 outputs (0.075 cyc/elem) vs the
  row-Toeplitz scheme's 21 per 108 (0.194) -- ~2.6x fewer streamed
  columns.  ty is processed in groups of 6 so each matmul streams
  N = 6*77 = 462 columns into one PSUM bank (462*4B < 2KB).

  Inputs/weights bf16 (PSUM accumulates fp32), output stored bf16 and
  upcast on host; end-to-end rel err ~3e-3.
"""
import sys

sys.path.insert(0, "/opt/trn_rl_repo")

import numpy as np
import ml_dtypes

N_CORES = 8
B, C, H, W = 16, 3, 768, 768
KS = 21          # kernel size
PAD = 10         # reflect pad
BH, BW = 12, 10  # space-to-depth block shape
PB = BH * BW     # 120 partitions / outputs per block
PYB, PXB = 66, 79   # padded-image block grid (792 x 790, zero-extended)
TY, TX = 64, 77     # output block grid (768 rows exact, 770 cols -> crop)
NOFF = 9            # 3x3 block offsets
OFFS = [(a, b) for a in range(3) for b in range(3)]
TGRP = [(t, 6) for t in range(0, 60, 6)] + [(60, 4)]  # ty groups
SPC = B // N_CORES  # samples per core = 2
IMGS = SPC * C      # images per core = 6

_prog_cache = {}


def build_program(reps=1, loop_reps=1):
    """loop_reps>1 wraps the whole conv in a hardware For_i loop repeating it
    loop_reps times -- used only for timing (constant instruction count)."""
    import contextlib

    import concourse.bacc as bacc
    import concourse.mybir as mybir
    from concourse.tile import TileContext

    nc = bacc.Bacc(None, target_bir_lowering=False)
    x = nc.declare_dram_parameter("x", [IMGS, PB, PYB, PXB], mybir.dt.bfloat16,
                                  isOutput=False)
    w = nc.declare_dram_parameter("w", [PB, SPC * NOFF, PB], mybir.dt.bfloat16,
                                  isOutput=False)
    y = nc.declare_dram_parameter("y", [IMGS, PB, TY, TX], mybir.dt.bfloat16,
                                  isOutput=True)

    with TileContext(nc) as tc:
        with (
            tc.tile_pool(name="wpool", bufs=1) as wpool,
            tc.tile_pool(name="xpool", bufs=3) as xpool,
            tc.tile_pool(name="opool", bufs=4) as opool,
            tc.tile_pool(name="psum", bufs=8, space="PSUM") as psum_pool,
        ):
            w_sb = wpool.tile([PB, SPC * NOFF, PB], mybir.dt.bfloat16)
            nc.sync.dma_start(out=w_sb[:, :, :], in_=w[:, :, :])

            loop_cm = (
                tc.For_i(0, loop_reps, 1) if loop_reps > 1
                else contextlib.nullcontext()
            )
            with loop_cm:
                for _ in range(reps):
                    for img in range(IMGS):
                        s = img // C
                        x_sb = xpool.tile([PB, PYB, PXB], mybir.dt.bfloat16)
                        nc.sync.dma_start(out=x_sb[:, :, :], in_=x[img])
                        for ty0, t in TGRP:
                            ps = psum_pool.tile([PB, 6, TX], mybir.dt.float32)
                            for j, (al, be) in enumerate(OFFS):
                                nc.tensor.matmul(
                                    ps[:, :t, :],
                                    w_sb[:, s * NOFF + j, :],
                                    x_sb[:, ty0 + al : ty0 + al + t,
                                         be : be + TX],
                                    start=(j == 0),
                                    stop=(j == NOFF - 1),
                                )
                            out_sb = opool.tile([PB, 6, TX],
                                                mybir.dt.bfloat16)
                            nc.vector.tensor_copy(out=out_sb[:, :t, :],
                                                  in_=ps[:, :t, :])
                            nc.sync.dma_start(
                                out=y[img, :, ty0 : ty0 + t, :],
                                in_=out_sb[:, :t, :],
                            )
    nc.compile()
    return nc


def _weights(kern_pair):
    """kern_pair [SPC, 21, 21] -> w [PB, SPC*NOFF, PB] bf16 stationaries."""
    ap, bp = np.divmod(np.arange(PB), BW)
    dy0 = ap[:, None] - ap[None, :]   # a' - a
    dx0 = bp[:, None] - bp[None, :]   # b' - b
    wt = np.zeros((PB, SPC * NOFF, PB), np.float32)
    for s in range(SPC):
        k = kern_pair[s]
        for j, (al, be) in enumerate(OFFS):
            dy = BH * al + dy0
            dx = BW * be + dx0
            v = (dy >= 0) & (dy < KS) & (dx >= 0) & (dx < KS)
            wt[:, s * NOFF + j, :] = np.where(
                v, k[dy.clip(0, KS - 1), dx.clip(0, KS - 1)], 0.0)
    return wt.astype(ml_dtypes.bfloat16)


def make_in_maps(inp, kern):
    pad = np.pad(inp, ((0, 0), (0, 0), (PAD, PAD), (PAD, PAD)), mode="reflect")
    in_maps = []
    for c in range(N_CORES):
        s0 = c * SPC
        xc = pad[s0 : s0 + SPC].reshape(IMGS, H + 2 * PAD, W + 2 * PAD)
        ext = np.zeros((IMGS, PYB * BH, PXB * BW), np.float32)
        ext[:, : H + 2 * PAD, : W + 2 * PAD] = xc
        p2d = (ext.reshape(IMGS, PYB, BH, PXB, BW)
               .transpose(0, 2, 4, 1, 3)
               .reshape(IMGS, PB, PYB, PXB))
        in_maps.append({
            "x": np.ascontiguousarray(p2d.astype(ml_dtypes.bfloat16)),
            "w": _weights(kern[s0 : s0 + SPC]),
        })
    return in_maps


def kernel(input, kernel):
    from concourse.bass_utils import run_bass_kernel_spmd

    inp = np.asarray(input, dtype=np.float32)
    kern = np.asarray(kernel, dtype=np.float32)
    in_maps = make_in_maps(inp, kern)

    if "nc" not in _prog_cache:
        _prog_cache["nc"] = build_program()
    nc = _prog_cache["nc"]

    res = run_bass_kernel_spmd(nc, in_maps, list(range(N_CORES)))
    out = np.empty((B, C, H, W), np.float32)
    for c in range(N_CORES):
        yc = np.asarray(res.results[c]["y"], dtype=np.float32)  # [6,120,TY,TX]
        img = (yc.reshape(IMGS, BH, BW, TY, TX)
               .transpose(0, 3, 1, 4, 2)
               .reshape(IMGS, TY * BH, TX * BW)[:, :, :W])
        out[c * SPC : (c + 1) * SPC] = img.reshape(SPC, C, H, W)
    return out


# revision 8
# speedup vs baseline: 2.1064x; 1.0133x over previous
"""Per-sample 21x21 blur (grouped conv, reflect pad) on trn2, 8 NeuronCores.

Problem: input [16, 3, 768, 768] f32, kernel [16, 21, 21] f32 (one blur
kernel per sample, shared across channels), reflect-pad 10, output
[16, 3, 768, 768] f32.

Strategy (data-parallel over batch, 2 samples/core, 6 images/core):

  Space-to-depth matmul formulation.  The padded image (788x788, zero-
  extended to 792x790) is laid out host-side as 12x10 pixel blocks with
  the 120 in-block pixels on SBUF partitions:

      P2D[p=(a',b'), (py, px)] = pad[12*py + a', 10*px + b']

  An output block (ty, tx) needs pad rows [12ty, 12ty+32) and cols
  [10tx, 10tx+30), i.e. pad-blocks (ty+al, tx+be) for al, be in 0..2.
  The conv is then NINE accumulating matmuls with dense-ish 120x120
  stationaries, the moving operand being the natural P2D tensor at the
  nine block offsets -- no im2col, the layout swizzle is free (host):

      out[(a,b), (ty,tx)] = sum_{al,be} W_ab[(a',b'), (a,b)]
                                        * P2D[(a',b'), (ty+al, tx+be)]
      W_ab[(a',b'), (a,b)] = k[12*al + a' - a, 10*be + b' - b]

  PE cost: 9 streamed columns per 120 outputs (0.075 cyc/elem) vs the
  row-Toeplitz scheme's 21 per 108 (0.194) -- ~2.6x fewer streamed
  columns.  ty is processed in groups of 6 so each matmul streams
  N = 6*77 = 462 columns into one PSUM bank (462*4B < 2KB).

  Inputs/weights bf16 (PSUM accumulates fp32), output stored bf16 and
  upcast on host; end-to-end rel err ~3e-3.
"""
import sys

sys.path.insert(0, "/opt/trn_rl_repo")

import numpy as np
import ml_dtypes

N_CORES = 8
B, C, H, W = 16, 3, 768, 768
KS = 21          # kernel size
PAD = 10         # reflect pad
BH, BW = 12, 10  # space-to-depth block shape
PB = BH * BW     # 120 partitions / outputs per block
PYB, PXB = 66, 79   # padded-image block grid (792 x 790, zero-extended)
TY, TX = 64, 77     # output block grid (768 rows exact, 770 cols -> crop)
NOFF = 9            # 3x3 block offsets
OFFS = [(a, b) for a in range(3) for b in range(3)]
TGRP = [(t, 6) for t in range(0, 60, 6)] + [(60, 4)]  # ty groups
SPC = B // N_CORES  # samples per core = 2
IMGS = SPC * C      # images per core = 6

_prog_cache = {}


def build_program(reps=1, loop_reps=1):
    """loop_reps>1 wraps the whole conv in a hardware For_i loop repeating it
    loop_reps times -- used only for timing (constant instruction count)."""
    import contextlib

    import concourse.bacc as bacc
    import concourse.mybir as mybir
    from concourse.tile import TileContext

    nc = bacc.Bacc(None, target_bir_lowering=False)
    x = nc.declare_dram_parameter("x", [IMGS, 128, PYB, PXB], mybir.dt.bfloat16,
                                  isOutput=False)
    w = nc.declare_dram_parameter("w", [128, SPC * NOFF, 128], mybir.dt.bfloat16,
                                  isOutput=False)
    y = nc.declare_dram_parameter("y", [IMGS, PB, TY, TX], mybir.dt.bfloat16,
                                  isOutput=True)

    with TileContext(nc) as tc:
        with (
            tc.tile_pool(name="wpool", bufs=1) as wpool,
            tc.tile_pool(name="xpool", bufs=3) as xpool,
            tc.tile_pool(name="opool", bufs=4) as opool,
            tc.tile_pool(name="psum", bufs=8, space="PSUM") as psum_pool,
        ):
            w_sb = wpool.tile([128, SPC * NOFF, 128], mybir.dt.bfloat16)
            nc.sync.dma_start(out=w_sb[:, :, :], in_=w[:, :, :])

            loop_cm = (
                tc.For_i(0, loop_reps, 1) if loop_reps > 1
                else contextlib.nullcontext()
            )
            with loop_cm:
                for _ in range(reps):
                    for img in range(IMGS):
                        s = img // C
                        x_sb = xpool.tile([128, PYB, PXB], mybir.dt.bfloat16)
                        nc.sync.dma_start(out=x_sb[:, :, :], in_=x[img])
                        for ty0, t in TGRP:
                            ps = psum_pool.tile([128, 6, TX], mybir.dt.float32)
                            for j, (al, be) in enumerate(OFFS):
                                nc.tensor.matmul(
                                    ps[:, :t, :],
                                    w_sb[:, s * NOFF + j, :],
                                    x_sb[:, ty0 + al : ty0 + al + t,
                                         be : be + TX],
                                    start=(j == 0),
                                    stop=(j == NOFF - 1),
                                )
                            out_sb = opool.tile([PB, 6, TX],
                                                mybir.dt.bfloat16)
                            nc.vector.tensor_copy(out=out_sb[:, :t, :],
                                                  in_=ps[:PB, :t, :])
                            nc.sync.dma_start(
                                out=y[img, :, ty0 : ty0 + t, :],
                                in_=out_sb[:, :t, :],
                            )
    nc.compile()
    return nc


def _weights(kern_pair):
    """kern_pair [SPC, 21, 21] -> w [PB, SPC*NOFF, PB] bf16 stationaries."""
    ap, bp = np.divmod(np.arange(PB), BW)
    dy0 = ap[:, None] - ap[None, :]   # a' - a
    dx0 = bp[:, None] - bp[None, :]   # b' - b
    wt = np.zeros((128, SPC * NOFF, 128), np.float32)
    for s in range(SPC):
        k = kern_pair[s]
        for j, (al, be) in enumerate(OFFS):
            dy = BH * al + dy0
            dx = BW * be + dx0
            v = (dy >= 0) & (dy < KS) & (dx >= 0) & (dx < KS)
            wt[:PB, s * NOFF + j, :PB] = np.where(
                v, k[dy.clip(0, KS - 1), dx.clip(0, KS - 1)], 0.0)
    return wt.astype(ml_dtypes.bfloat16)


def make_in_maps(inp, kern):
    pad = np.pad(inp, ((0, 0), (0, 0), (PAD, PAD), (PAD, PAD)), mode="reflect")
    in_maps = []
    for c in range(N_CORES):
        s0 = c * SPC
        xc = pad[s0 : s0 + SPC].reshape(IMGS, H + 2 * PAD, W + 2 * PAD)
        ext = np.zeros((IMGS, PYB * BH, PXB * BW), np.float32)
        ext[:, : H + 2 * PAD, : W + 2 * PAD] = xc
        p2d = np.zeros((IMGS, 128, PYB, PXB), ml_dtypes.bfloat16)
        p2d[:, :PB] = (ext.reshape(IMGS, PYB, BH, PXB, BW)
                       .transpose(0, 2, 4, 1, 3)
                       .reshape(IMGS, PB, PYB, PXB))
        in_maps.append({
            "x": p2d,
            "w": _weights(kern[s0 : s0 + SPC]),
        })
    return in_maps


def kernel(input, kernel):
    from concourse.bass_utils import run_bass_kernel_spmd

    inp = np.asarray(input, dtype=np.float32)
    kern = np.asarray(kernel, dtype=np.float32)
    in_maps = make_in_maps(inp, kern)

    if "nc" not in _prog_cache:
        _prog_cache["nc"] = build_program()
    nc = _prog_cache["nc"]

    res = run_bass_kernel_spmd(nc, in_maps, list(range(N_CORES)))
    out = np.empty((B, C, H, W), np.float32)
    for c in range(N_CORES):
        yc = np.asarray(res.results[c]["y"], dtype=np.float32)  # [6,120,TY,TX]
        img = (yc.reshape(IMGS, BH, BW, TY, TX)
               .transpose(0, 3, 1, 4, 2)
               .reshape(IMGS, TY * BH, TX * BW)[:, :, :W])
        out[c * SPC : (c + 1) * SPC] = img.reshape(SPC, C, H, W)
    return out


# revision 9
# speedup vs baseline: 2.1560x; 1.0235x over previous
"""Per-sample 21x21 blur (grouped conv, reflect pad) on trn2, 8 NeuronCores.

Problem: input [16, 3, 768, 768] f32, kernel [16, 21, 21] f32 (one blur
kernel per sample, shared across channels), reflect-pad 10, output
[16, 3, 768, 768] f32.

Strategy (data-parallel over batch, 2 samples/core, 6 images/core):

  Space-to-depth matmul formulation.  The padded image (788x788, zero-
  extended to 792x790) is laid out host-side as 12x10 pixel blocks with
  the 120 in-block pixels on SBUF partitions:

      P2D[p=(a',b'), (py, px)] = pad[12*py + a', 10*px + b']

  An output block (ty, tx) needs pad rows [12ty, 12ty+32) and cols
  [10tx, 10tx+30), i.e. pad-blocks (ty+al, tx+be) for al, be in 0..2.
  The conv is then NINE accumulating matmuls with dense-ish 120x120
  stationaries, the moving operand being the natural P2D tensor at the
  nine block offsets -- no im2col, the layout swizzle is free (host):

      out[(a,b), (ty,tx)] = sum_{al,be} W_ab[(a',b'), (a,b)]
                                        * P2D[(a',b'), (ty+al, tx+be)]
      W_ab[(a',b'), (a,b)] = k[12*al + a' - a, 10*be + b' - b]

  PE cost: 9 streamed columns per 120 outputs (0.075 cyc/elem) vs the
  row-Toeplitz scheme's 21 per 108 (0.194) -- ~2.6x fewer streamed
  columns.  ty is processed in groups of 6 so each matmul streams
  N = 6*77 = 462 columns into one PSUM bank (462*4B < 2KB).

  Inputs/weights bf16 (PSUM accumulates fp32), output stored bf16 and
  upcast on host; end-to-end rel err ~3e-3.
"""
import sys

sys.path.insert(0, "/opt/trn_rl_repo")

import numpy as np
import ml_dtypes

N_CORES = 8
B, C, H, W = 16, 3, 768, 768
KS = 21          # kernel size
PAD = 10         # reflect pad
BH, BW = 12, 10  # space-to-depth block shape
PB = BH * BW     # 120 partitions / outputs per block
PYB, PXB = 66, 79   # padded-image block grid (792 x 790, zero-extended)
TY, TX = 64, 77     # output block grid (768 rows exact, 770 cols -> crop)
NOFF = 9            # 3x3 block offsets
OFFS = [(a, b) for a in range(3) for b in range(3)]
TGRP = [(t, 6) for t in range(0, 60, 6)] + [(60, 4)]  # ty groups
SPC = B // N_CORES  # samples per core = 2
IMGS = SPC * C      # images per core = 6

_prog_cache = {}


def build_program(reps=1, loop_reps=1):
    """loop_reps>1 wraps the whole conv in a hardware For_i loop repeating it
    loop_reps times -- used only for timing (constant instruction count)."""
    import contextlib

    import concourse.bacc as bacc
    import concourse.mybir as mybir
    from concourse.tile import TileContext

    nc = bacc.Bacc(None, target_bir_lowering=False)
    x = nc.declare_dram_parameter("x", [IMGS, 128, PYB, PXB], mybir.dt.bfloat16,
                                  isOutput=False)
    w = nc.declare_dram_parameter("w", [128, SPC * NOFF, 128], mybir.dt.bfloat16,
                                  isOutput=False)
    y = nc.declare_dram_parameter("y", [IMGS, PB, TY, TX], mybir.dt.bfloat16,
                                  isOutput=True)

    with TileContext(nc) as tc:
        with (
            tc.tile_pool(name="wpool", bufs=1) as wpool,
            tc.tile_pool(name="xpool", bufs=3) as xpool,
            tc.tile_pool(name="opool", bufs=4) as opool,
            tc.tile_pool(name="psum", bufs=8, space="PSUM") as psum_pool,
        ):
            w_sb = wpool.tile([128, SPC * NOFF, 128], mybir.dt.bfloat16)
            nc.sync.dma_start(out=w_sb[:, :, :], in_=w[:, :, :])

            loop_cm = (
                tc.For_i(0, loop_reps, 1) if loop_reps > 1
                else contextlib.nullcontext()
            )
            with loop_cm:
                for _ in range(reps):
                    for img in range(IMGS):
                        s = img // C
                        x_sb = xpool.tile([128, PYB, PXB], mybir.dt.bfloat16)
                        nc.sync.dma_start(out=x_sb[:, :, :], in_=x[img])
                        for ty0, t in TGRP:
                            # Flat PSUM with per-matmul offset (2-be): the
                            # whole t*79-wide moving slab streams as one
                            # contiguous (1-level) AP; every 79-elem row
                            # lands be-shifted so all 9 offsets accumulate
                            # aligned at q = 2 + 79*tyl + tx.
                            ps = psum_pool.tile([128, 480], mybir.dt.float32)
                            for j, (al, be) in enumerate(OFFS):
                                nc.tensor.matmul(
                                    ps[:, 2 - be : 2 - be + t * PXB],
                                    w_sb[:, s * NOFF + j, :],
                                    x_sb[:, ty0 + al : ty0 + al + t, :],
                                    start=(j == 0),
                                    stop=(j == NOFF - 1),
                                )
                            out_sb = opool.tile([PB, 6, TX],
                                                mybir.dt.bfloat16)
                            src = (ps[:PB, 2 : 2 + t * PXB]
                                   .rearrange("p (a b) -> p a b", b=PXB)
                                   [:, :, :TX])
                            nc.vector.tensor_copy(out=out_sb[:, :t, :],
                                                  in_=src)
                            nc.sync.dma_start(
                                out=y[img, :, ty0 : ty0 + t, :],
                                in_=out_sb[:, :t, :],
                            )
    nc.compile()
    return nc


def _weights(kern_pair):
    """kern_pair [SPC, 21, 21] -> w [PB, SPC*NOFF, PB] bf16 stationaries."""
    ap, bp = np.divmod(np.arange(PB), BW)
    dy0 = ap[:, None] - ap[None, :]   # a' - a
    dx0 = bp[:, None] - bp[None, :]   # b' - b
    wt = np.zeros((128, SPC * NOFF, 128), np.float32)
    for s in range(SPC):
        k = kern_pair[s]
        for j, (al, be) in enumerate(OFFS):
            dy = BH * al + dy0
            dx = BW * be + dx0
            v = (dy >= 0) & (dy < KS) & (dx >= 0) & (dx < KS)
            wt[:PB, s * NOFF + j, :PB] = np.where(
                v, k[dy.clip(0, KS - 1), dx.clip(0, KS - 1)], 0.0)
    return wt.astype(ml_dtypes.bfloat16)


def make_in_maps(inp, kern):
    pad = np.pad(inp, ((0, 0), (0, 0), (PAD, PAD), (PAD, PAD)), mode="reflect")
    in_maps = []
    for c in range(N_CORES):
        s0 = c * SPC
        xc = pad[s0 : s0 + SPC].reshape(IMGS, H + 2 * PAD, W + 2 * PAD)
        ext = np.zeros((IMGS, PYB * BH, PXB * BW), np.float32)
        ext[:, : H + 2 * PAD, : W + 2 * PAD] = xc
        p2d = np.zeros((IMGS, 128, PYB, PXB), ml_dtypes.bfloat16)
        p2d[:, :PB] = (ext.reshape(IMGS, PYB, BH, PXB, BW)
                       .transpose(0, 2, 4, 1, 3)
                       .reshape(IMGS, PB, PYB, PXB))
        in_maps.append({
            "x": p2d,
            "w": _weights(kern[s0 : s0 + SPC]),
        })
    return in_maps


def kernel(input, kernel):
    from concourse.bass_utils import run_bass_kernel_spmd

    inp = np.asarray(input, dtype=np.float32)
    kern = np.asarray(kernel, dtype=np.float32)
    in_maps = make_in_maps(inp, kern)

    if "nc" not in _prog_cache:
        _prog_cache["nc"] = build_program()
    nc = _prog_cache["nc"]

    res = run_bass_kernel_spmd(nc, in_maps, list(range(N_CORES)))
    out = np.empty((B, C, H, W), np.float32)
    for c in range(N_CORES):
        yc = np.asarray(res.results[c]["y"], dtype=np.float32)  # [6,120,TY,TX]
        img = (yc.reshape(IMGS, BH, BW, TY, TX)
               .transpose(0, 3, 1, 4, 2)
               .reshape(IMGS, TY * BH, TX * BW)[:, :, :W])
        out[c * SPC : (c + 1) * SPC] = img.reshape(SPC, C, H, W)
    return out


# revision 13
# speedup vs baseline: 2.1620x; 1.0028x over previous
"""Per-sample 21x21 blur (grouped conv, reflect pad) on trn2, 8 NeuronCores.

Problem: input [16, 3, 768, 768] f32, kernel [16, 21, 21] f32 (one blur
kernel per sample, shared across channels), reflect-pad 10, output
[16, 3, 768, 768] f32.

Strategy (data-parallel over batch, 2 samples/core, 6 images/core):

  Space-to-depth matmul formulation.  The padded image (788x788, zero-
  extended to 792x790) is laid out host-side as 12x10 pixel blocks with
  the 120 in-block pixels on SBUF partitions:

      P2D[p=(a',b'), (py, px)] = pad[12*py + a', 10*px + b']

  An output block (ty, tx) needs pad rows [12ty, 12ty+32) and cols
  [10tx, 10tx+30), i.e. pad-blocks (ty+al, tx+be) for al, be in 0..2.
  The conv is then NINE accumulating matmuls with dense-ish 120x120
  stationaries, the moving operand being the natural P2D tensor at the
  nine block offsets -- no im2col, the layout swizzle is free (host):

      out[(a,b), (ty,tx)] = sum_{al,be} W_ab[(a',b'), (a,b)]
                                        * P2D[(a',b'), (ty+al, tx+be)]
      W_ab[(a',b'), (a,b)] = k[12*al + a' - a, 10*be + b' - b]

  PE cost: 9 streamed columns per 120 outputs (0.075 cyc/elem) vs the
  row-Toeplitz scheme's 21 per 108 (0.194) -- ~2.6x fewer streamed
  columns.  ty is processed in groups of 6 so each matmul streams
  N = 6*77 = 462 columns into one PSUM bank (462*4B < 2KB).

  Inputs/weights bf16 (PSUM accumulates fp32), output stored bf16 and
  upcast on host; end-to-end rel err ~3e-3.
"""
import sys

sys.path.insert(0, "/opt/trn_rl_repo")

import numpy as np
import ml_dtypes

N_CORES = 8
B, C, H, W = 16, 3, 768, 768
KS = 21          # kernel size
PAD = 10         # reflect pad
BH, BW = 12, 10  # space-to-depth block shape
PB = BH * BW     # 120 partitions / outputs per block
PYB, PXB = 66, 79   # padded-image block grid (792 x 790, zero-extended)
TY, TX = 64, 77     # output block grid (768 rows exact, 770 cols -> crop)
NOFF = 9            # 3x3 block offsets
OFFS = [(a, b) for a in range(3) for b in range(3)]
TGRP = [(t, 6) for t in range(0, 60, 6)] + [(60, 4)]  # ty groups
SPC = B // N_CORES  # samples per core = 2
IMGS = SPC * C      # images per core = 6

_prog_cache = {}


def build_program(reps=1, loop_reps=1):
    """loop_reps>1 wraps the whole conv in a hardware For_i loop repeating it
    loop_reps times -- used only for timing (constant instruction count)."""
    import contextlib

    import concourse.bacc as bacc
    import concourse.mybir as mybir
    from concourse.tile import TileContext

    nc = bacc.Bacc(None, target_bir_lowering=False)
    x = nc.declare_dram_parameter("x", [IMGS, 128, PYB * PXB], mybir.dt.bfloat16,
                                  isOutput=False)
    w = nc.declare_dram_parameter("w", [128, SPC * NOFF, 128], mybir.dt.bfloat16,
                                  isOutput=False)
    y = nc.declare_dram_parameter("y", [IMGS, PB, TY, TX], mybir.dt.bfloat16,
                                  isOutput=True)

    with TileContext(nc) as tc:
        with (
            tc.tile_pool(name="wpool", bufs=1) as wpool,
            tc.tile_pool(name="xpool", bufs=3) as xpool,
            tc.tile_pool(name="opool", bufs=4) as opool,
            tc.tile_pool(name="psum", bufs=8, space="PSUM") as psum_pool,
        ):
            w_sb = wpool.tile([128, SPC * NOFF, 128], mybir.dt.bfloat16)
            nc.sync.dma_start(out=w_sb[:, :, :], in_=w[:, :, :])

            loop_cm = (
                tc.For_i(0, loop_reps, 1) if loop_reps > 1
                else contextlib.nullcontext()
            )
            with loop_cm:
                for _ in range(reps):
                    for img in range(IMGS):
                        s = img // C
                        # Flat x slab with a 2-elem front pad: element
                        # (py, px) sits at flat 2 + 79*py + px.  Each matmul
                        # streams one contiguous 1-level slice at base
                        # 79*(ty0+al) + be and writes PSUM at the FIXED
                        # range [0, t*79) -- so only ONE instruction field
                        # (the rhs base) varies across the 9 offsets, which
                        # keeps the per-matmul issue cost minimal.  The
                        # alignment works out to psum q = 2 + 79*tyl + tx
                        # accumulating x[ty0+al+tyl, tx+be] for all 9.
                        x_sb = xpool.tile([128, 2 + PYB * PXB],
                                          mybir.dt.bfloat16)
                        nc.sync.dma_start(
                            out=x_sb[:, 2 : 2 + PYB * PXB],
                            in_=x[img],
                        )
                        for ty0, t in TGRP:
                            ps = psum_pool.tile([128, 478], mybir.dt.float32)
                            for j, (al, be) in enumerate(OFFS):
                                base = PXB * (ty0 + al) + be
                                nc.tensor.matmul(
                                    ps[:, 0 : t * PXB],
                                    w_sb[:, s * NOFF + j, :],
                                    x_sb[:, base : base + t * PXB],
                                    start=(j == 0),
                                    stop=(j == NOFF - 1),
                                )
                            out_sb = opool.tile([PB, 6, TX],
                                                mybir.dt.bfloat16)
                            src = (ps[:PB, 2 : 2 + t * PXB]
                                   .rearrange("p (a b) -> p a b", b=PXB)
                                   [:, :, :TX])
                            nc.vector.tensor_copy(out=out_sb[:, :t, :],
                                                  in_=src)
                            nc.sync.dma_start(
                                out=y[img, :, ty0 : ty0 + t, :],
                                in_=out_sb[:, :t, :],
                            )
    nc.compile()
    return nc


def _weights(kern_pair):
    """kern_pair [SPC, 21, 21] -> w [PB, SPC*NOFF, PB] bf16 stationaries."""
    ap, bp = np.divmod(np.arange(PB), BW)
    dy0 = ap[:, None] - ap[None, :]   # a' - a
    dx0 = bp[:, None] - bp[None, :]   # b' - b
    wt = np.zeros((128, SPC * NOFF, 128), np.float32)
    for s in range(SPC):
        k = kern_pair[s]
        for j, (al, be) in enumerate(OFFS):
            dy = BH * al + dy0
            dx = BW * be + dx0
            v = (dy >= 0) & (dy < KS) & (dx >= 0) & (dx < KS)
            wt[:PB, s * NOFF + j, :PB] = np.where(
                v, k[dy.clip(0, KS - 1), dx.clip(0, KS - 1)], 0.0)
    return wt.astype(ml_dtypes.bfloat16)


def make_in_maps(inp, kern):
    pad = np.pad(inp, ((0, 0), (0, 0), (PAD, PAD), (PAD, PAD)), mode="reflect")
    in_maps = []
    for c in range(N_CORES):
        s0 = c * SPC
        xc = pad[s0 : s0 + SPC].reshape(IMGS, H + 2 * PAD, W + 2 * PAD)
        ext = np.zeros((IMGS, PYB * BH, PXB * BW), np.float32)
        ext[:, : H + 2 * PAD, : W + 2 * PAD] = xc
        p2d = np.zeros((IMGS, 128, PYB * PXB), ml_dtypes.bfloat16)
        p2d[:, :PB] = (ext.reshape(IMGS, PYB, BH, PXB, BW)
                       .transpose(0, 2, 4, 1, 3)
                       .reshape(IMGS, PB, PYB * PXB))
        in_maps.append({
            "x": p2d,
            "w": _weights(kern[s0 : s0 + SPC]),
        })
    return in_maps


def kernel(input, kernel):
    from concourse.bass_utils import run_bass_kernel_spmd

    inp = np.asarray(input, dtype=np.float32)
    kern = np.asarray(kernel, dtype=np.float32)
    in_maps = make_in_maps(inp, kern)

    if "nc" not in _prog_cache:
        _prog_cache["nc"] = build_program()
    nc = _prog_cache["nc"]

    res = run_bass_kernel_spmd(nc, in_maps, list(range(N_CORES)))
    out = np.empty((B, C, H, W), np.float32)
    for c in range(N_CORES):
        yc = np.asarray(res.results[c]["y"], dtype=np.float32)  # [6,120,TY,TX]
        img = (yc.reshape(IMGS, BH, BW, TY, TX)
               .transpose(0, 3, 1, 4, 2)
               .reshape(IMGS, TY * BH, TX * BW)[:, :, :W])
        out[c * SPC : (c + 1) * SPC] = img.reshape(SPC, C, H, W)
    return out
